# revision 15
# baseline (speedup 1.0000x reference)
"""Trainium2 Bass kernel for nn_Beltrami (retrieval_knn).

Per-core (batch-parallel over 8 cores):
  fc (f32r hi/lo-split matmuls) -> normalize pos -> sim in 4 matmuls per
  512-chunk: 2x f16 hi*hi (scale 2^8 each side) + 2x e4m3 DoubleRow
  cross-terms (hi at 2^3, lo at 2^13; all products at 2^16) -> E =
  exp(psum * 2^-16) fp32 straight from PSUM (ranking by E, monotone in
  sim) -> top-32 via 24-chunk max8 + 4-round refine on E -> mask chain:
  D = f16(E - E32) on Act, M01 = (D >= 0) on DVE (4x), A = E*M01 on Pool
  -> A^T via DRAM round-trip -> f16 gather matmul A @ [feat|1] ->
  reciprocal-normalized output.
"""
import sys
import numpy as np

sys.path.insert(0, "/opt/trn_rl_repo")

B, N, C, K = 8, 4096, 256, 32
NT = N // 128          # 32 query tiles of 128 rows
NEG = -1.0e30

# selection scan chunks: 16 of 170 + 8 of 172 = 4096
CHUNKS = [170] * 16 + [172] * 8
NCH = len(CHUNKS)

_CACHE = {}


def _build(reps=1):
    from contextlib import ExitStack
    import concourse.bass as bass
    import concourse.bacc as bacc
    import concourse.tile as tile
    from concourse import mybir

    f32 = mybir.dt.float32
    f32r = mybir.dt.float32r
    f16 = mybir.dt.float16
    f8 = mybir.dt.float8e4
    AF = mybir.ActivationFunctionType
    Alu = mybir.AluOpType
    DR = mybir.MatmulPerfMode.DoubleRow

    nc = bacc.Bacc("TRN2", target_bir_lowering=False, debug=False, num_devices=8)

    xT_in = nc.declare_dram_parameter("xT", [C, N], f32, isOutput=False)
    wT_in = nc.declare_dram_parameter("wT", [C, 2 * C], f32, isOutput=False)
    bf_in = nc.declare_dram_parameter("bf", [1, 2 * C], f32, isOutput=False)
    bp_in = nc.declare_dram_parameter("bp", [128, 2], f32, isOutput=False)
    id_in = nc.declare_dram_parameter("ident", [128, 128], f32, isOutput=False)
    out_p = nc.declare_dram_parameter("out", [N, C], f32, isOutput=True)
    a_drams = [nc.dram_tensor(f"a_scratch{i}", [128, N], f16) for i in range(4)]
    s_dram = nc.dram_tensor("s_scratch", [NT, 128], f32)

    with tile.TileContext(nc) as tc, ExitStack() as ctx:
        # ---------------- persistent pools ----------------
        persist = ctx.enter_context(tc.tile_pool(name="persist", bufs=1))
        featx_pool = ctx.enter_context(tc.tile_pool(name="featx", bufs=NT))

        ph = [persist.tile([128, N], f16, tag=f"ph{ct}", name=f"ph{ct}") for ct in range(2)]
        h8 = persist.tile([128, 2, N], f8, tag="h8")
        l8 = persist.tile([128, 2, N], f8, tag="l8")
        featx = [featx_pool.tile([128, C + 2], f16, tag="featx", name=f"featx{i}") for i in range(NT)]
        nrm2 = persist.tile([128, NT], f32, tag="nrm2")
        s_til = persist.tile([128, NT], f32, tag="s_til")

        # ---------------- startup: fc + normalize + quantize ----------------
        with ExitStack() as sctx:
            sb = sctx.enter_context(tc.tile_pool(name="start_sb", bufs=1))
            ps_fc = sctx.enter_context(tc.tile_pool(name="ps_fc", bufs=3, space="PSUM"))
            ps_pp = sctx.enter_context(tc.tile_pool(name="ps_pp", bufs=3, space="PSUM"))
            ps_tp = sctx.enter_context(tc.tile_pool(name="ps_tp", bufs=1, space="PSUM"))

            xstage_pool = sctx.enter_context(tc.tile_pool(name="xstage_pool", bufs=3))
            lo_pool = sctx.enter_context(tc.tile_pool(name="lo_pool", bufs=3))
            xt_r = [sb.tile([128, N], f32r, tag=f"xt_r{ct}", name=f"xt_r{ct}") for ct in range(2)]
            wt = [sb.tile([128, 2 * C], f32, tag=f"wt{ct}", name=f"wt{ct}") for ct in range(2)]
            wt_r = [sb.tile([128, C], f32r, tag=f"wt_r{ct}", name=f"wt_r{ct}") for ct in range(2)]
            xt_lo = [sb.tile([128, N], f32r, tag=f"xt_lo{ct}", name=f"xt_lo{ct}") for ct in range(2)]
            wph = [sb.tile([128, C], f32r, tag=f"wph{ct}", name=f"wph{ct}") for ct in range(2)]
            wpl = [sb.tile([128, C], f32r, tag=f"wpl{ct}", name=f"wpl{ct}") for ct in range(2)]
            bf1_r = sb.tile([1, 2 * C], f32r, tag="bf1_r")
            ones1_r = sb.tile([1, 128], f32r, tag="ones1_r")
            bf1 = sb.tile([1, 2 * C], f32, tag="bf1")
            bp = sb.tile([128, 2], f32, tag="bp")
            ones1 = sb.tile([1, 128], f32, tag="ones1")
            scrap = sb.tile([128, C], f16, tag="scrap")
            post_raw = [sb.tile([128, N], f32, tag=f"post_raw{ct}", name=f"post_raw{ct}") for ct in range(2)]

            for ct in range(2):
                nc.sync.dma_start(wt[ct][:], wT_in[ct * 128:(ct + 1) * 128, :])
            nc.sync.dma_start(bf1[:], bf_in[:])
            nc.sync.dma_start(bp[:], bp_in[:])
            nc.vector.memset(ones1[:], 1.0)
            for ct in range(2):
                nc.vector.tensor_copy(wt_r[ct][:], wt[ct][:, 0:C])
                nc.vector.tensor_copy(wph[ct][:], wt[ct][:, C:2 * C])
                nc.vector.tensor_tensor(wpl[ct][:], wt[ct][:, C:2 * C],
                                        wph[ct][:], op=Alu.subtract)
            nc.vector.tensor_copy(bf1_r[:], bf1[:])
            nc.vector.tensor_copy(ones1_r[:], ones1[:])

            # staging + fc interleaved per 512-chunk so PE stays dense
            def stage_chunk(ch):
                cs = slice(ch * 512, (ch + 1) * 512)
                for ct in range(2):
                    xstage = xstage_pool.tile([128, 512], f32, tag="xstage",
                                              name=f"xstage{ct}_{ch}")
                    nc.sync.dma_start(xstage[:], xT_in[ct * 128:(ct + 1) * 128, cs])
                    nc.vector.tensor_copy(xt_r[ct][:, cs], xstage[:])
                    nc.vector.tensor_tensor(xt_lo[ct][:, cs], xstage[:],
                                            xt_r[ct][:, cs], op=Alu.subtract)

            def posT_chunk(dt, ch):
                pp = ps_pp.tile([128, 512], f32, tag="pp", name=f"pp{dt}_{ch}")
                ds_ = slice(dt * 128, (dt + 1) * 128)
                cs_ = slice(ch * 512, (ch + 1) * 512)
                for ci, (lh, rh) in enumerate(
                        [(wph[0], xt_r[0]), (wph[0], xt_lo[0]), (wpl[0], xt_r[0]),
                         (wph[1], xt_r[1]), (wph[1], xt_lo[1]), (wpl[1], xt_r[1])]):
                    nc.tensor.matmul(pp[:], lh[:, ds_], rh[:, cs_],
                                     start=(ci == 0), stop=(ci == 5))
                nc.scalar.activation(
                    post_raw[dt][:, ch * 512:(ch + 1) * 512], pp[:],
                    AF.Identity, bias=bp[:, dt:dt + 1])

            for ch in range(8):
                stage_chunk(ch)
            for nt in range(NT):
                fc = ps_fc.tile([128, 2 * C], f32, tag="fc")
                ns = slice(nt * 128, (nt + 1) * 128)
                nc.tensor.matmul(fc[:, 0:C], xt_r[0][:, ns], wt_r[0][:],
                                 start=True, stop=False)
                nc.tensor.matmul(fc[:, 0:C], xt_r[1][:, ns], wt_r[1][:],
                                 start=False, stop=False)
                nc.tensor.matmul(fc[:, 0:C], ones1_r[:], bf1_r[:, 0:C],
                                 start=False, stop=True)
                for ci, (lh, rh) in enumerate(
                        [(xt_r[0], wph[0]), (xt_r[0], wpl[0]), (xt_lo[0], wph[0]),
                         (xt_r[1], wph[1]), (xt_r[1], wpl[1]), (xt_lo[1], wph[1])]):
                    nc.tensor.matmul(fc[:, C:2 * C], lh[:, ns], rh[:],
                                     start=(ci == 0), stop=False)
                nc.tensor.matmul(fc[:, C:2 * C], ones1_r[:], bf1_r[:, C:2 * C],
                                 start=False, stop=True)
                nc.vector.tensor_copy(featx[nt][:, 0:C], fc[:, 0:C])
                nc.gpsimd.memset(featx[nt][:, C:C + 1], 1.0)
                nc.gpsimd.memset(featx[nt][:, C + 1:C + 2], 0.0)
                nc.scalar.activation(scrap[:], fc[:, C:2 * C], AF.Square,
                                     accum_out=nrm2[:, nt:nt + 1])

            for ch in range(8):
                posT_chunk(0, ch)
                posT_chunk(1, ch)

            # rsqrt of norms with two Newton steps
            r0 = sb.tile([128, NT], f32, tag="r0")
            u = sb.tile([128, NT], f32, tag="u")
            nc.vector.reciprocal(r0[:], nrm2[:])
            nc.scalar.activation(s_til[:], r0[:], AF.Sqrt)
            for _ in range(2):
                nc.vector.tensor_tensor(u[:], s_til[:], s_til[:], op=Alu.mult)
                nc.vector.tensor_tensor(u[:], u[:], nrm2[:], op=Alu.mult)
                nc.vector.tensor_scalar(u[:], u[:], -0.5, scalar2=1.5,
                                        op0=Alu.mult, op1=Alu.add)
                nc.vector.tensor_tensor(s_til[:], s_til[:], u[:], op=Alu.mult)

            # transpose s [128, NT] -> [NT, 128], bounce via DRAM, broadcast-load
            ident = sb.tile([128, 128], f32, tag="ident")
            nc.sync.dma_start(ident[:], id_in[:])
            st_ps = ps_tp.tile([NT, 128], f32, tag="st_ps")
            nc.tensor.transpose(st_ps[:], s_til[:], ident[:])
            stt = sb.tile([NT, 128], f32, tag="stt")
            nc.vector.tensor_copy(stt[:], st_ps[:])
            nc.sync.dma_start(s_dram[:], stt[:])

            # normalize + quantize: p = post_raw * s ; ph=f16(p*256),
            # h8=e4m3(p*8), l8=e4m3((p-hi)*8192)
            for ch in range(8):
                cs = slice(ch * 512, (ch + 1) * 512)
                sbc = xstage_pool.tile([128, 512], f32, tag="sbc",
                                       name=f"sbc{ch}")
                nc.sync.dma_start(
                    sbc[:], s_dram[:].flatten()[cs].partition_broadcast(128))
                for ct in range(2):
                    lo = lo_pool.tile([128, 512], f32, tag="lo",
                                      name=f"lo{ct}_{ch}")
                    nc.gpsimd.tensor_tensor(post_raw[ct][:, cs],
                                            post_raw[ct][:, cs],
                                            sbc[:], op=Alu.mult)
                    nc.scalar.mul(ph[ct][:, cs], post_raw[ct][:, cs], 256.0)
                    nc.vector.tensor_scalar(h8[:, ct, cs], post_raw[ct][:, cs],
                                            8.0, scalar2=None, op0=Alu.mult)
                    nc.vector.scalar_tensor_tensor(
                        lo[:], ph[ct][:, cs], -(2.0 ** -8),
                        post_raw[ct][:, cs], op0=Alu.mult, op1=Alu.add)
                    nc.scalar.mul(l8[:, ct, cs], lo[:], 8192.0)

        # ---------------- steady loop: software-pipelined, depth 3 ----------
        e_pool = ctx.enter_context(tc.tile_pool(name="e_sb", bufs=4))
        i_pool = ctx.enter_context(tc.tile_pool(name="i_sb", bufs=5))
        at_pool = ctx.enter_context(tc.tile_pool(name="at_sb", bufs=4))
        cands_pool = ctx.enter_context(tc.tile_pool(name="cands_sb", bufs=2))
        bi_pool = ctx.enter_context(tc.tile_pool(name="bi_sb", bufs=3))
        osb_pool = ctx.enter_context(tc.tile_pool(name="osb_sb", bufs=2))
        ps_sim = ctx.enter_context(tc.tile_pool(name="ps_sim", bufs=3, space="PSUM"))
        ps_oe = ctx.enter_context(tc.tile_pool(name="ps_oe", bufs=2, space="PSUM"))

        Es, As, ATs, bIs = {}, {}, {}, {}

        def issue_sim_hk(T, hk):
            E = Es[T]
            qs = slice(T * 128, (T + 1) * 128)
            pp = ps_sim.tile([128, 1024], f32, tag="pp", name=f"pp{T}_{hk}")
            for sub in range(2):
                cs = slice(hk * 1024 + sub * 512, hk * 1024 + (sub + 1) * 512)
                os_ = slice(sub * 512, (sub + 1) * 512)
                nc.tensor.matmul(pp[:, os_], ph[0][:, qs], ph[0][:, cs],
                                 start=True, stop=False)
                nc.tensor.matmul(pp[:, os_], ph[1][:, qs], ph[1][:, cs],
                                 start=False, stop=False)
                nc.tensor.matmul(pp[:, os_], h8[:, :, qs], l8[:, :, cs],
                                 start=False, stop=False, perf_mode=DR)
                nc.tensor.matmul(pp[:, os_], l8[:, :, qs], h8[:, :, cs],
                                 start=False, stop=True, perf_mode=DR)
            nc.scalar.activation(E[:, hk * 1024:(hk + 1) * 1024], pp[:],
                                 AF.Exp, scale=2.0 ** -16)

        def issue_D_half(T, h):
            # Act: D = f16(E - E32), sign-exact compare operand
            hs = slice(h * 2048, (h + 1) * 2048)
            nc.scalar.activation(As[T][:, hs], Es[T][:, hs], AF.Identity,
                                 bias=bIs[T][:, 0:1], scale=1.0)

        def issue_scan_part(T, c0, c1):
            E, cands = Es[T], candss[T]
            off = sum(CHUNKS[:c0])
            for c in range(c0, c1):
                w = CHUNKS[c]
                nc.vector.max(cands[:, c * 8:(c + 1) * 8], E[:, off:off + w])
                off += w

        def issue_M01_mult_half(T, h):
            # DVE: M01 = (D >= 0) in place; Pool: A = E * M01; DMA store
            E, A = Es[T], As[T]
            hs = slice(h * 2048, (h + 1) * 2048)
            nc.vector.tensor_scalar(A[:, hs], A[:, hs], 0.0, scalar2=None,
                                    op0=Alu.is_ge)
            nc.gpsimd.tensor_tensor(A[:, hs], E[:, hs], A[:, hs],
                                    op=Alu.mult)
            nc.sync.dma_start(a_drams[T % 4][:, hs], A[:, hs])

        def issue_transposes(T):
            AT = ATs[T]
            for h in range(2):
                hs = slice(h * 2048, (h + 1) * 2048)
                nc.sync.dma_start_transpose(
                    AT[:, h * 16:(h + 1) * 16, :], a_drams[T % 4][:, hs])

        def issue_refine(T, cands):
            r8 = cands_pool.tile([128, 8], f32, tag="r8", name=f"r8{T}")
            for rnd in range(4):
                nc.vector.max(r8[:], cands[:])
                if rnd < 3:
                    nc.vector.match_replace(out=cands[:], in_to_replace=r8[:],
                                            in_values=cands[:], imm_value=NEG)
            bI = bi_pool.tile([128, 1], f32, tag="bI", name=f"bI{T}")
            nc.vector.tensor_scalar(bI[:], r8[:, 7:8], -1.0, scalar2=None,
                                    op0=Alu.mult)
            bIs[T] = bI

        def issue_gather(T):
            AT = ATs.pop(T)
            Es.pop(T); As.pop(T); bIs.pop(T)
            oe = ps_oe.tile([128, C + 2], f32, tag="oe", name=f"oe{T}")
            for j in range(NT):
                nc.tensor.matmul(oe[:], AT[:, j, :], featx[j][:],
                                 start=(j == 0), stop=(j == NT - 1))
            rz = bi_pool.tile([128, 1], f32, tag="rz", name=f"rz{T}")
            nc.vector.reciprocal(rz[:], oe[:, C:C + 1])
            osb = osb_pool.tile([128, C], f32, tag="osb", name=f"osb{T}")
            nc.scalar.activation(osb[:], oe[:, 0:C], AF.Copy, scale=rz[:])
            nc.sync.dma_start(out_p[T * 128:(T + 1) * 128, :], osb[:])

        LAG = 5
        candss = {}
        for rep in range(reps):
            for T in range(NT + LAG):
                sim_live = T < NT
                mask_live = 2 <= T <= NT + 1
                scan_live = 1 <= T <= NT
                if sim_live:
                    Es[T] = e_pool.tile([128, N], f32, tag="E", name=f"E{T}")
                if scan_live:
                    candss[T - 1] = cands_pool.tile(
                        [128, 8 * NCH], f32, tag="cands", name=f"cands{T-1}")
                if mask_live:
                    As[T - 2] = i_pool.tile([128, N], f16, tag="A",
                                            name=f"A{T - 2}")
                    ATs[T - 2] = at_pool.tile([128, NT, 128], f16, tag="AT",
                                              name=f"AT{T - 2}")
                    issue_D_half(T - 2, 0)
                if sim_live:
                    issue_sim_hk(T, 0)
                if mask_live:
                    issue_D_half(T - 2, 1)
                if sim_live:
                    for hk in range(1, 4):
                        issue_sim_hk(T, hk)
                if scan_live:
                    issue_scan_part(T - 1, 0, 12)
                if mask_live:
                    issue_M01_mult_half(T - 2, 0)
                if scan_live:
                    issue_scan_part(T - 1, 12, NCH)
                if mask_live:
                    issue_M01_mult_half(T - 2, 1)
                    issue_transposes(T - 2)
                if scan_live:
                    issue_refine(T - 1, candss.pop(T - 1))
                if LAG <= T:
                    issue_gather(T - LAG)

    nc.compile()
    return nc


def kernel(x, W, bias, k):
    from concourse.bass_utils import run_bass_kernel_spmd

    x = np.asarray(x, dtype=np.float32)
    W = np.asarray(W, dtype=np.float32)
    bias = np.asarray(bias, dtype=np.float32)
    assert int(k) == K and x.shape == (B, N, C)

    if "nc" not in _CACHE:
        _CACHE["nc"] = _build()
    nc = _CACHE["nc"]

    wT = np.ascontiguousarray(W.T)                      # [C, 2C]
    bf = bias.reshape(1, 2 * C)
    bp = np.ascontiguousarray(
        bias[C:].reshape(2, 128).T)                     # [128, 2]
    ident = np.eye(128, dtype=np.float32)

    in_maps = []
    for b in range(B):
        xT = np.ascontiguousarray(x[b].T)               # [C, N]
        in_maps.append({"xT": xT, "wT": wT, "bf": bf, "bp": bp,
                        "ident": ident})

    res = run_bass_kernel_spmd(nc, in_maps, list(range(B)))
    out = np.stack([res.results[b]["out"] for b in range(B)], axis=0)
    return out.astype(np.float32)
